# revision 19
# baseline (speedup 1.0000x reference)
"""Multi-head attention, tensor-parallel over heads x data-parallel over batch.

8 NeuronCores: core c handles batch b=c//2, head-group g=c%2 (8 heads, 512 chans).
Each core computes its head-group's attention + partial output projection;
the two partials per batch are summed on the host (row-parallel Wo unshard).

v3: all matmul operands bf16 (fp32 PSUM accumulation), with the schedule built
around keeping TensorE dense while ScalarE streams the 64 exp ACTIVATEs:
  - input DMA split across three engine queues (sync / gpsimd / scalar)
  - pair-0 q/k projections k-streamed behind the DMA into two score-psum tiles
  - PE prewarmed with discarded matmuls so the HAM clock-gate opens early
  - v-projection + later pairs' q/k projections interleaved into attention
    as PE filler; out-projection token-blocks 0-3 fill pair-3's second half
  - AV stationaries are [ones | v] per head so the softmax denominators
    accumulate for free on partitions 0:64 of the same psum bank
  - normalization fully on DVE: reciprocal_approx_fast at base partition 0
    (required), cross-partition muls against the PSUM operand
"""

import numpy as np
import ml_dtypes

import concourse.bacc as bacc
import concourse.mybir as mybir
import concourse.tile as tile
from concourse import bass_utils

B = 4
T = 1024          # tokens (N = L)
D = 1024          # model dim
CH = 64           # channels per head
G = 512           # channels per head-group (8 heads)
SCALE = CH ** -0.5
NEG = -30000.0    # mask bias (exp(x + NEG) == 0)
F32 = mybir.dt.float32
BF16 = mybir.dt.bfloat16
NPBF16 = ml_dtypes.bfloat16

N_CORES = 8
KB = 8            # 128-row contraction blocks over D
TB = 8            # 128-token blocks
PAIRS = 4         # head pairs per core
VW = 256          # v-tile columns per head pair: [ones | v_even | ones | v_odd]
AV_LAG = 3        # software-pipeline depth: AV trails scores/exp by this many jb

LAST_RESULTS = None
_CACHE = {}


def _emit(tc):
    nc = tc.nc
    xqT = nc.dram_tensor("xqT", [D, T], BF16, kind="ExternalInput").ap()
    xkvT = nc.dram_tensor("xkvT", [D, T], BF16, kind="ExternalInput").ap()
    wq = nc.dram_tensor("wq", [D, G], BF16, kind="ExternalInput").ap()
    wk = nc.dram_tensor("wk", [D, G], BF16, kind="ExternalInput").ap()
    wv = nc.dram_tensor("wv", [D, G], BF16, kind="ExternalInput").ap()
    wo = nc.dram_tensor("wo", [G, D], BF16, kind="ExternalInput").ap()
    mb = nc.dram_tensor("mb", [128, TB], F32, kind="ExternalInput").ap()
    vones = nc.dram_tensor("vones", [128, PAIRS * CH], BF16,
                           kind="ExternalInput").ap()
    out = nc.dram_tensor("out", [T, D], BF16, kind="ExternalOutput").ap()

    Exp = mybir.ActivationFunctionType.Exp

    with (
        tc.tile_pool(name="wpool", bufs=1) as wpool,
        tc.tile_pool(name="xpool", bufs=1) as xpool,
        tc.tile_pool(name="actpool", bufs=1) as actpool,
        tc.tile_pool(name="etpool", bufs=16) as etpool,
        tc.tile_pool(name="rpool", bufs=2) as rpool,
        tc.tile_pool(name="opool", bufs=2) as opool,
        tc.tile_pool(name="psum_s", bufs=2, space="PSUM") as psum_s,
        tc.tile_pool(name="psum_o", bufs=1, space="PSUM") as psum_o,
        tc.tile_pool(name="psum_p", bufs=2, space="PSUM") as psum_p,
    ):
        # ------------- input DMA, split across engine queues -------------
        # critical path first on every queue (everything the pair-0 q/k
        # k-stream needs: xq, wq, wk, xkv); wv and wo trail afterwards.
        mask_t = wpool.tile([128, TB], F32, name="mask_t", tag="mask")
        nc.scalar.dma_start(mask_t[:], mb[:])
        ones_t = wpool.tile([128, PAIRS * CH], BF16, name="ones_t", tag="ones")
        nc.scalar.dma_start(ones_t[:], vones[:])

        xq_t = [xpool.tile([128, T], BF16, name=f"xq{k}", tag=f"xq{k}")
                for k in range(KB)]
        wq_t = [wpool.tile([128, G], BF16, name=f"wq{k}", tag=f"wq{k}")
                for k in range(KB)]
        xkv_t = [xpool.tile([128, T], BF16, name=f"xkv{k}", tag=f"xkv{k}")
                 for k in range(KB)]
        # scalar queue: xq6-7, xkv6-7, wk, then (late) wo
        for k in (6, 7):
            nc.scalar.dma_start(xq_t[k][:], xqT[k * 128:(k + 1) * 128, :])
            nc.scalar.dma_start(xkv_t[k][:], xkvT[k * 128:(k + 1) * 128, :])
        wk_t = []
        for k in range(KB):
            t2 = wpool.tile([128, G], BF16, name=f"wk{k}", tag=f"wk{k}")
            nc.scalar.dma_start(t2[:], wk[k * 128:(k + 1) * 128, :])
            wk_t.append(t2)

        # sync queue: xkv0-5 interleaved with wq
        for k in range(6):
            nc.sync.dma_start(xkv_t[k][:], xkvT[k * 128:(k + 1) * 128, :])
            nc.sync.dma_start(wq_t[k][:], wq[k * 128:(k + 1) * 128, :])
        for k in (6, 7):
            nc.sync.dma_start(wq_t[k][:], wq[k * 128:(k + 1) * 128, :])

        # gpsimd queue: xq0-5, then wv (needed by the v-unit filler early)
        for k in range(6):
            nc.gpsimd.dma_start(xq_t[k][:], xqT[k * 128:(k + 1) * 128, :])
        wv_t = []
        for k in range(KB):
            t5 = wpool.tile([128, G], BF16, name=f"wv{k}", tag=f"wv{k}")
            nc.gpsimd.dma_start(t5[:], wv[k * 128:(k + 1) * 128, :])
            wv_t.append(t5)

        wo_t = []
        for m in range(PAIRS):
            t6 = wpool.tile([128, D], BF16, name=f"wo{m}", tag=f"wo{m}")
            nc.scalar.dma_start(t6[:], wo[m * 128:(m + 1) * 128, :])
            wo_t.append(t6)

        qT_t = [actpool.tile([128, T], BF16, name=f"qT{m}", tag=f"qT{m}")
                for m in range(PAIRS)]
        kT_t = [actpool.tile([128, T], BF16, name=f"kT{m}", tag=f"kT{m}")
                for m in range(PAIRS)]
        v_t = [actpool.tile([128, PAIRS * VW], BF16, name=f"v{tb}",
                            tag=f"v{tb}") for tb in range(TB)]
        oT_t = [actpool.tile([128, T], BF16, name=f"oT{m}", tag=f"oT{m}")
                for m in range(PAIRS)]

        # dummy-matmul target: the po psum slot, overwritten (start=True)
        # by the first real AV matmul
        warm = psum_o.tile([128, 512], F32, name="warm", tag="po")
        # ones blocks of the v tiles, copied from SBUF (not strided DMA)
        for tb in range(TB):
            v4 = v_t[tb].rearrange("p (pb four c) -> p pb four c",
                                   four=4, c=CH)
            vo3 = ones_t.rearrange("p (pb c) -> p pb c", c=CH)
            nc.vector.tensor_copy(v4[:, :, 0, :], vo3[:])
            nc.vector.tensor_copy(v4[:, :, 2, :], vo3[:])

        # ------- pair-0 q/k projections, k-streamed behind the DMA -------
        psA = psum_s.tile([128, 1024], F32, name="psA", tag="sc")
        psB = psum_s.tile([128, 1024], F32, name="psB", tag="sc")
        for k in range(KB):
            for ic in range(2):
                csl = slice(ic * 512, (ic + 1) * 512)
                nc.tensor.matmul(psA[:, csl], wq_t[k][:, 0:128],
                                 xq_t[k][:, csl],
                                 start=(k == 0), stop=(k == KB - 1))
                nc.tensor.matmul(psB[:, csl], wk_t[k][:, 0:128],
                                 xkv_t[k][:, csl],
                                 start=(k == 0), stop=(k == KB - 1))
            # prewarm the PE while the prologue is DMA-bound: junk matmuls
            # into the warm slot open the HAM clock gate (~3.4us of activity)
            for _ in range(2):
                nc.tensor.matmul(warm[:], wq_t[k][:, 0:128],
                                 xq_t[k][:, 0:512], start=True, stop=True)
        nc.vector.tensor_copy(qT_t[0][:, 0:512], psA[:, 0:512])
        nc.vector.tensor_copy(qT_t[0][:, 512:1024], psA[:, 512:1024])
        nc.vector.tensor_copy(kT_t[0][:, 0:512], psB[:, 0:512])
        nc.vector.tensor_copy(kT_t[0][:, 512:1024], psB[:, 512:1024])

        # ---------------- filler units (run inside attention) ----------------
        def v_unit(tb):
            """v = xkv @ Wv for one token block: 8 MMs + 1 copy."""
            ps = psum_p.tile([128, 512], F32, name="ps_v", tag="pj")
            for k in range(KB):
                nc.tensor.matmul(
                    ps[:],
                    xkv_t[k][:, tb * 128:(tb + 1) * 128],
                    wv_t[k][:],
                    start=(k == 0),
                    stop=(k == KB - 1),
                )
                yield
            v4 = v_t[tb].rearrange("p (pb four c) -> p pb four c",
                                   four=4, c=CH)
            ps3 = ps.rearrange("p (pb two c) -> p pb two c", two=2, c=CH)
            nc.vector.tensor_copy(v4[:, :, 1:2, :], ps3[:, :, 0:1, :])
            nc.vector.tensor_copy(v4[:, :, 3:4, :], ps3[:, :, 1:2, :])
            yield

        def qk_unit(p, src, w_t, dst, ic):
            csl = slice(ic * 512, (ic + 1) * 512)
            ps = psum_p.tile([128, 512], F32, name="ps_qk", tag="pj")
            for k in range(KB):
                nc.tensor.matmul(
                    ps[:],
                    w_t[k][:, p * 128:(p + 1) * 128],
                    src[k][:, csl],
                    start=(k == 0),
                    stop=(k == KB - 1),
                )
                yield
            nc.vector.tensor_copy(dst[p][:, csl], ps[:])
            yield

        def out_unit(tb):
            """out[tb] = oT.T @ Wo: 8 MMs + copy + DMA."""
            tsl = slice(tb * 128, (tb + 1) * 128)
            ps = psum_s.tile([128, 1024], F32, name="ps_out", tag="sc")
            for ncx in range(2):
                nsl = slice(ncx * 512, (ncx + 1) * 512)
                for m in range(PAIRS):
                    nc.tensor.matmul(
                        ps[:, nsl],
                        oT_t[m][:, tsl],
                        wo_t[m][:, nsl],
                        start=(m == 0),
                        stop=(m == PAIRS - 1),
                    )
                    yield
            ot = opool.tile([128, 1024], BF16, name="ot", tag="ot")
            nc.vector.tensor_copy(ot[:], ps[:])
            # spread the tail output blocks over otherwise-idle queues
            eng = (nc.scalar, nc.scalar, nc.scalar, nc.scalar,
                   nc.sync, nc.gpsimd, nc.sync, nc.gpsimd)[tb]
            eng.dma_start(out[tsl, :], ot[:])
            yield

        def chain(gens):
            for g in gens:
                yield from g

        # v stream (AV deadlines) and one qk stream per later pair.
        # v units are tracked per-unit so an AV is never emitted before the
        # v tile it reads has been fully emitted (program order = dataflow).
        vunits = [v_unit(tb) for tb in range(TB)]
        vstate = {"i": 0}

        def vstep(n):
            k = 0
            while k < n and vstate["i"] < TB:
                if next(vunits[vstate["i"]], "done") == "done":
                    vstate["i"] += 1
                else:
                    k += 1

        qkgens = {
            p: chain([qk_unit(p, src, w_t, dst, ic)
                      for src, w_t, dst in ((xkv_t, wk_t, kT_t),
                                            (xq_t, wq_t, qT_t))
                      for ic in range(2)])
            for p in range(1, PAIRS)
        }
        # out-projection blocks 0-3 only need ih=0 columns of oT: they fill
        # pair 3's second-half attention
        late_filler = chain([out_unit(tb) for tb in range(4)])

        def drain(gen, n):
            for _ in range(n):
                if next(gen, "done") == "done":
                    return

        # ---------------- attention ----------------
        def _av(item, p, po):
            jb, et = item
            for h in (0, 1):
                # head 2p+h stationary [ones | v]: sums land on partitions
                # 0:64, o on 64:128; one accumulation group per psum bank
                csl = slice(p * VW + h * 128, p * VW + h * 128 + 128)
                nc.tensor.matmul(
                    po[:, h * 512:(h + 1) * 512],
                    v_t[jb][:, csl],
                    et[:, h * 512:(h + 1) * 512],
                    start=(jb == 0),
                    stop=(jb == TB - 1),
                )

        def _norm(np_, nih, npo):
            # normalize. po bank0 rows = [s_h0 | o_h0], bank1 = [s_h1 | o_h1].
            # recip must run at base partition 0 (custom DVE ops misread at
            # other bases); the muls mix partition bases, allowed because one
            # operand is PSUM. oT rows: 0:64 = head 2p+1, 64:128 = head 2p
            # (host swaps Wo rows to match).
            noT = oT_t[np_]
            nisl = slice(nih * 512, (nih + 1) * 512)
            r_e = rpool.tile([64, 512], F32, name="r_e", tag="re")
            nc.vector.reciprocal_approx_fast(r_e[:], npo[0:64, 0:512])
            nc.vector.tensor_mul(noT[64:128, nisl], npo[64:128, 0:512],
                                 r_e[:])
            r_o = rpool.tile([64, 512], F32, name="r_o", tag="ro")
            nc.vector.reciprocal_approx_fast(r_o[:], npo[0:64, 512:1024])
            nc.vector.tensor_mul(noT[0:64, nisl], npo[64:128, 512:1024],
                                 r_o[:])

        def _force_av(item, ap, apo):
            while vstate["i"] <= item[0]:
                vstep(4)
            _av(item, ap, apo)

        # the trailing AV groups + normalization of each iteration are
        # carried into the next iteration's first jb slots so the exp
        # stream never waits behind the drain at iteration boundaries
        carry = None
        for p in range(PAIRS):
            for ih in range(2):
                isl = slice(ih * 512, (ih + 1) * 512)
                po = psum_o.tile([128, 1024], F32, name="po", tag="po")
                pend = []
                for jb in range(TB):
                    jsl = slice(jb * 128, (jb + 1) * 128)
                    pss = psum_s.tile([128, 1024], F32, name="ps_s", tag="sc")
                    for h in (0, 1):
                        hsl = slice(h * 64, (h + 1) * 64)
                        nc.tensor.matmul(
                            pss[:, h * 512:(h + 1) * 512],
                            kT_t[p][hsl, jsl],
                            qT_t[p][hsl, isl],
                        )
                    # one exp for both heads; mask bias is per-partition (= j)
                    et = etpool.tile([128, 1024], BF16, name="et", tag="et")
                    nc.scalar.activation(et[:], pss[:], Exp,
                                         bias=mask_t[:, jb:jb + 1],
                                         scale=SCALE)
                    pend.append((jb, et))
                    if carry is not None:
                        cpend, cp, cpo, cih = carry
                        _force_av(cpend.pop(0), cp, cpo)
                        if not cpend:
                            _norm(cp, cih, cpo)
                            carry = None
                    elif len(pend) > AV_LAG:
                        _force_av(pend.pop(0), p, po)
                    vstep(2)
                    # late_filler reads oT columns normed by the carried
                    # p3-ih0 norm: only drain once the carry has flushed
                    if p == PAIRS - 1 and ih == 1 and carry is None:
                        drain(late_filler, 2)
                    elif p < PAIRS - 1:
                        drain(qkgens[p + 1], 2)
                carry = (pend, p, po, ih)
                if ih == 1 and p < PAIRS - 1:
                    # next pair's projections must precede its scores
                    drain(qkgens[p + 1], 100)
        # final iteration's trailing AVs + norm
        cpend, cp, cpo, cih = carry
        while cpend:
            _force_av(cpend.pop(0), cp, cpo)
        _norm(cp, cih, cpo)

        # ---------------- remaining out-projection blocks ----------------
        drain(late_filler, 100)
        for tb in range(4, TB):
            for _ in out_unit(tb):
                pass


def build_nc():
    nc = bacc.Bacc("TRN2", target_bir_lowering=False, debug=False,
                   num_devices=N_CORES)
    with tile.TileContext(nc) as tc:
        _emit(tc)
    nc.compile()
    return nc


def _get_compiled():
    if "nc" not in _CACHE:
        _CACHE["nc"] = build_nc()
    return _CACHE["nc"]


def make_in_maps(x_q, x_kv, pad_mask):
    ones = np.ones((128, PAIRS * CH), NPBF16)
    in_maps = []
    for c in range(N_CORES):
        b, g = divmod(c, 2)
        gs = slice(g * G, (g + 1) * G)
        mbias = np.where(pad_mask[b], np.float32(NEG), np.float32(0.0))
        in_maps.append({
            "xqT": np.ascontiguousarray(x_q[b].T).astype(NPBF16),
            "xkvT": np.ascontiguousarray(x_kv[b].T).astype(NPBF16),
            "wq": _W["q"][:, gs].astype(NPBF16),
            "wk": _W["k"][:, gs].astype(NPBF16),
            "wv": _W["v"][:, gs].astype(NPBF16),
            # oT pair blocks are [head 2p+1 | head 2p]; swap Wo rows
            "wo": _W["o"][gs, :].reshape(PAIRS, 2, CH, D)[:, ::-1]
                  .reshape(G, D).astype(NPBF16),
            "mb": np.ascontiguousarray(
                mbias.astype(np.float32).reshape(TB, 128).T),
            "vones": ones,
        })
    return in_maps


_W = {}


def kernel(x_q, x_kv, pad_mask, Wq, Wk, Wv, Wo, bo):
    global LAST_RESULTS
    x_q = np.asarray(x_q, dtype=np.float32)
    x_kv = np.asarray(x_kv, dtype=np.float32)
    pad_mask = np.asarray(pad_mask)
    _W["q"] = np.asarray(Wq, dtype=np.float32)
    _W["k"] = np.asarray(Wk, dtype=np.float32)
    _W["v"] = np.asarray(Wv, dtype=np.float32)
    _W["o"] = np.asarray(Wo, dtype=np.float32)
    bo = np.asarray(bo, dtype=np.float32)

    nc = _get_compiled()
    in_maps = make_in_maps(x_q, x_kv, pad_mask)
    res = bass_utils.run_bass_kernel_spmd(nc, in_maps, list(range(N_CORES)))
    LAST_RESULTS = res
    outp = np.zeros((B, T, D), np.float32)
    for b in range(B):
        outp[b] = (res.results[2 * b]["out"].astype(np.float32)
                   + res.results[2 * b + 1]["out"].astype(np.float32))
    outp += bo[None, None, :]
    return outp


# revision 20
# speedup vs baseline: 1.1609x; 1.1609x over previous
"""Multi-head attention, tensor-parallel over heads x data-parallel over batch.

8 NeuronCores: core c handles batch b=c//2, head-group g=c%2 (8 heads, 512 chans).
Each core computes its head-group's attention + partial output projection;
the two partials per batch are summed on the host (row-parallel Wo unshard).

v3: all matmul operands bf16 (fp32 PSUM accumulation), with the schedule built
around keeping TensorE dense while ScalarE streams the 64 exp ACTIVATEs:
  - input DMA split across three engine queues (sync / gpsimd / scalar)
  - pair-0 q/k projections k-streamed behind the DMA into two score-psum tiles
  - PE prewarmed with discarded matmuls so the HAM clock-gate opens early
  - v-projection + later pairs' q/k projections interleaved into attention
    as PE filler; out-projection token-blocks 0-3 fill pair-3's second half
  - AV stationaries are [ones | v] per head so the softmax denominators
    accumulate for free on partitions 0:64 of the same psum bank
  - normalization fully on DVE: reciprocal_approx_fast at base partition 0
    (required), cross-partition muls against the PSUM operand
"""

import numpy as np
import ml_dtypes

import concourse.bacc as bacc
import concourse.mybir as mybir
import concourse.tile as tile
from concourse import bass_utils

B = 4
T = 1024          # tokens (N = L)
D = 1024          # model dim
CH = 64           # channels per head
G = 512           # channels per head-group (8 heads)
SCALE = CH ** -0.5
NEG = -30000.0    # mask bias (exp(x + NEG) == 0)
F32 = mybir.dt.float32
BF16 = mybir.dt.bfloat16
NPBF16 = ml_dtypes.bfloat16

N_CORES = 8
KB = 8            # 128-row contraction blocks over D
TB = 8            # 128-token blocks
PAIRS = 4         # head pairs per core
VW = 256          # v-tile columns per head pair: [ones | v_even | ones | v_odd]
AV_LAG = 3        # software-pipeline depth: AV trails scores/exp by this many jb

LAST_RESULTS = None
_CACHE = {}


def _emit(tc):
    nc = tc.nc
    xqT = nc.dram_tensor("xqT", [D, T], BF16, kind="ExternalInput").ap()
    xkvT = nc.dram_tensor("xkvT", [D, T], BF16, kind="ExternalInput").ap()
    wq = nc.dram_tensor("wq", [D, G], BF16, kind="ExternalInput").ap()
    wk = nc.dram_tensor("wk", [D, G], BF16, kind="ExternalInput").ap()
    wv = nc.dram_tensor("wv", [D, G], BF16, kind="ExternalInput").ap()
    wo = nc.dram_tensor("wo", [G, D], BF16, kind="ExternalInput").ap()
    mb = nc.dram_tensor("mb", [128, TB], F32, kind="ExternalInput").ap()
    vones = nc.dram_tensor("vones", [128, PAIRS * CH], BF16,
                           kind="ExternalInput").ap()
    out = nc.dram_tensor("out", [T, D], BF16, kind="ExternalOutput").ap()

    Exp = mybir.ActivationFunctionType.Exp

    with (
        tc.tile_pool(name="wpool", bufs=1) as wpool,
        tc.tile_pool(name="xpool", bufs=1) as xpool,
        tc.tile_pool(name="actpool", bufs=1) as actpool,
        tc.tile_pool(name="etpool", bufs=16) as etpool,
        tc.tile_pool(name="rpool", bufs=2) as rpool,
        tc.tile_pool(name="opool", bufs=2) as opool,
        tc.tile_pool(name="psum_s", bufs=2, space="PSUM") as psum_s,
        tc.tile_pool(name="psum_o", bufs=1, space="PSUM") as psum_o,
        tc.tile_pool(name="psum_p", bufs=2, space="PSUM") as psum_p,
    ):
        # ------------- input DMA, split across engine queues -------------
        # critical path first on every queue (everything the pair-0 q/k
        # k-stream needs: xq, wq, wk, xkv); wv and wo trail afterwards.
        mask_t = wpool.tile([128, TB], F32, name="mask_t", tag="mask")
        nc.scalar.dma_start(mask_t[:], mb[:])
        ones_t = wpool.tile([128, PAIRS * CH], BF16, name="ones_t", tag="ones")
        nc.scalar.dma_start(ones_t[:], vones[:])

        xq_t = [xpool.tile([128, T], BF16, name=f"xq{k}", tag=f"xq{k}")
                for k in range(KB)]
        wq_t = [wpool.tile([128, G], BF16, name=f"wq{k}", tag=f"wq{k}")
                for k in range(KB)]
        # scalar queue: last two xq blocks, then wk, then (late) wo
        for k in (6, 7):
            nc.scalar.dma_start(xq_t[k][:], xqT[k * 128:(k + 1) * 128, :])
        wk_t = []
        for k in range(KB):
            t2 = wpool.tile([128, G], BF16, name=f"wk{k}", tag=f"wk{k}")
            nc.scalar.dma_start(t2[:], wk[k * 128:(k + 1) * 128, :])
            wk_t.append(t2)

        # sync queue: xkv, then (late) wv
        xkv_t = []
        for k in range(KB):
            t4 = xpool.tile([128, T], BF16, name=f"xkv{k}", tag=f"xkv{k}")
            nc.sync.dma_start(t4[:], xkvT[k * 128:(k + 1) * 128, :])
            xkv_t.append(t4)

        # gpsimd queue: xq blocks 0-5 interleaved with wq
        for k in range(6):
            nc.gpsimd.dma_start(xq_t[k][:], xqT[k * 128:(k + 1) * 128, :])
            nc.gpsimd.dma_start(wq_t[k][:], wq[k * 128:(k + 1) * 128, :])
        for k in (6, 7):
            nc.gpsimd.dma_start(wq_t[k][:], wq[k * 128:(k + 1) * 128, :])

        # trailing, non-critical: wv (sync), wo (scalar)
        wv_t = []
        for k in range(KB):
            t5 = wpool.tile([128, G], BF16, name=f"wv{k}", tag=f"wv{k}")
            nc.sync.dma_start(t5[:], wv[k * 128:(k + 1) * 128, :])
            wv_t.append(t5)
        wo_t = []
        for m in range(PAIRS):
            t6 = wpool.tile([128, D], BF16, name=f"wo{m}", tag=f"wo{m}")
            nc.scalar.dma_start(t6[:], wo[m * 128:(m + 1) * 128, :])
            wo_t.append(t6)

        qT_t = [actpool.tile([128, T], BF16, name=f"qT{m}", tag=f"qT{m}")
                for m in range(PAIRS)]
        kT_t = [actpool.tile([128, T], BF16, name=f"kT{m}", tag=f"kT{m}")
                for m in range(PAIRS)]
        v_t = [actpool.tile([128, PAIRS * VW], BF16, name=f"v{tb}",
                            tag=f"v{tb}") for tb in range(TB)]
        oT_t = [actpool.tile([128, T], BF16, name=f"oT{m}", tag=f"oT{m}")
                for m in range(PAIRS)]

        # dummy-matmul target: the po psum slot, overwritten (start=True)
        # by the first real AV matmul
        warm = psum_o.tile([128, 512], F32, name="warm", tag="po")
        # ones blocks of the v tiles, copied from SBUF (not strided DMA)
        for tb in range(TB):
            v4 = v_t[tb].rearrange("p (pb four c) -> p pb four c",
                                   four=4, c=CH)
            vo3 = ones_t.rearrange("p (pb c) -> p pb c", c=CH)
            nc.vector.tensor_copy(v4[:, :, 0, :], vo3[:])
            nc.vector.tensor_copy(v4[:, :, 2, :], vo3[:])

        # ------- pair-0 q/k projections, k-streamed behind the DMA -------
        psA = psum_s.tile([128, 1024], F32, name="psA", tag="sc")
        psB = psum_s.tile([128, 1024], F32, name="psB", tag="sc")
        for k in range(KB):
            for ic in range(2):
                csl = slice(ic * 512, (ic + 1) * 512)
                nc.tensor.matmul(psA[:, csl], wq_t[k][:, 0:128],
                                 xq_t[k][:, csl],
                                 start=(k == 0), stop=(k == KB - 1))
                nc.tensor.matmul(psB[:, csl], wk_t[k][:, 0:128],
                                 xkv_t[k][:, csl],
                                 start=(k == 0), stop=(k == KB - 1))
            # prewarm the PE while the prologue is DMA-bound: junk matmuls
            # into the warm slot open the HAM clock gate (~3.4us of activity)
            for _ in range(2):
                nc.tensor.matmul(warm[:], wq_t[k][:, 0:128],
                                 xq_t[k][:, 0:512], start=True, stop=True)
        nc.vector.tensor_copy(qT_t[0][:, 0:512], psA[:, 0:512])
        nc.vector.tensor_copy(qT_t[0][:, 512:1024], psA[:, 512:1024])
        nc.vector.tensor_copy(kT_t[0][:, 0:512], psB[:, 0:512])
        nc.vector.tensor_copy(kT_t[0][:, 512:1024], psB[:, 512:1024])

        # ---------------- filler units (run inside attention) ----------------
        def v_unit(tb):
            """v = xkv @ Wv for one token block: 8 MMs + 1 copy."""
            ps = psum_p.tile([128, 512], F32, name="ps_v", tag="pj")
            for k in range(KB):
                nc.tensor.matmul(
                    ps[:],
                    xkv_t[k][:, tb * 128:(tb + 1) * 128],
                    wv_t[k][:],
                    start=(k == 0),
                    stop=(k == KB - 1),
                )
                yield
            v4 = v_t[tb].rearrange("p (pb four c) -> p pb four c",
                                   four=4, c=CH)
            ps3 = ps.rearrange("p (pb two c) -> p pb two c", two=2, c=CH)
            nc.vector.tensor_copy(v4[:, :, 1:2, :], ps3[:, :, 0:1, :])
            nc.vector.tensor_copy(v4[:, :, 3:4, :], ps3[:, :, 1:2, :])
            yield

        def qk_unit(p, src, w_t, dst, ic):
            csl = slice(ic * 512, (ic + 1) * 512)
            ps = psum_p.tile([128, 512], F32, name="ps_qk", tag="pj")
            for k in range(KB):
                nc.tensor.matmul(
                    ps[:],
                    w_t[k][:, p * 128:(p + 1) * 128],
                    src[k][:, csl],
                    start=(k == 0),
                    stop=(k == KB - 1),
                )
                yield
            nc.vector.tensor_copy(dst[p][:, csl], ps[:])
            yield

        def out_unit(tb):
            """out[tb] = oT.T @ Wo: 8 MMs + copy + DMA."""
            tsl = slice(tb * 128, (tb + 1) * 128)
            ps = psum_s.tile([128, 1024], F32, name="ps_out", tag="sc")
            for ncx in range(2):
                nsl = slice(ncx * 512, (ncx + 1) * 512)
                for m in range(PAIRS):
                    nc.tensor.matmul(
                        ps[:, nsl],
                        oT_t[m][:, tsl],
                        wo_t[m][:, nsl],
                        start=(m == 0),
                        stop=(m == PAIRS - 1),
                    )
                    yield
            ot = opool.tile([128, 1024], BF16, name="ot", tag="ot")
            nc.vector.tensor_copy(ot[:], ps[:])
            # spread the tail output blocks over otherwise-idle queues
            eng = (nc.scalar, nc.scalar, nc.scalar, nc.scalar,
                   nc.sync, nc.gpsimd, nc.sync, nc.gpsimd)[tb]
            eng.dma_start(out[tsl, :], ot[:])
            yield

        def chain(gens):
            for g in gens:
                yield from g

        # v stream (AV deadlines) and one qk stream per later pair.
        # v units are tracked per-unit so an AV is never emitted before the
        # v tile it reads has been fully emitted (program order = dataflow).
        vunits = [v_unit(tb) for tb in range(TB)]
        vstate = {"i": 0}

        def vstep(n):
            k = 0
            while k < n and vstate["i"] < TB:
                if next(vunits[vstate["i"]], "done") == "done":
                    vstate["i"] += 1
                else:
                    k += 1

        qkgens = {
            p: chain([qk_unit(p, src, w_t, dst, ic)
                      for src, w_t, dst in ((xkv_t, wk_t, kT_t),
                                            (xq_t, wq_t, qT_t))
                      for ic in range(2)])
            for p in range(1, PAIRS)
        }
        # out-projection blocks 0-3 only need ih=0 columns of oT: they fill
        # pair 3's second-half attention
        late_filler = chain([out_unit(tb) for tb in range(4)])

        def drain(gen, n):
            for _ in range(n):
                if next(gen, "done") == "done":
                    return

        # ---------------- attention ----------------
        def _av(item, p, po):
            jb, et = item
            for h in (0, 1):
                # head 2p+h stationary [ones | v]: sums land on partitions
                # 0:64, o on 64:128; one accumulation group per psum bank
                csl = slice(p * VW + h * 128, p * VW + h * 128 + 128)
                nc.tensor.matmul(
                    po[:, h * 512:(h + 1) * 512],
                    v_t[jb][:, csl],
                    et[:, h * 512:(h + 1) * 512],
                    start=(jb == 0),
                    stop=(jb == TB - 1),
                )

        for p in range(PAIRS):
            oT = oT_t[p]
            for ih in range(2):
                isl = slice(ih * 512, (ih + 1) * 512)
                po = psum_o.tile([128, 1024], F32, name="po", tag="po")
                pend = []
                for jb in range(TB):
                    jsl = slice(jb * 128, (jb + 1) * 128)
                    pss = psum_s.tile([128, 1024], F32, name="ps_s", tag="sc")
                    for h in (0, 1):
                        hsl = slice(h * 64, (h + 1) * 64)
                        nc.tensor.matmul(
                            pss[:, h * 512:(h + 1) * 512],
                            kT_t[p][hsl, jsl],
                            qT_t[p][hsl, isl],
                        )
                    # one exp for both heads; mask bias is per-partition (= j)
                    et = etpool.tile([128, 1024], BF16, name="et", tag="et")
                    nc.scalar.activation(et[:], pss[:], Exp,
                                         bias=mask_t[:, jb:jb + 1],
                                         scale=SCALE)
                    pend.append((jb, et))
                    if len(pend) > AV_LAG:
                        item = pend.pop(0)
                        while vstate["i"] <= item[0]:
                            vstep(4)
                        _av(item, p, po)
                    vstep(3)
                    if p == PAIRS - 1 and ih == 1:
                        drain(late_filler, 2)
                    elif p < PAIRS - 1:
                        drain(qkgens[p + 1], 2)
                while pend:
                    item = pend.pop(0)
                    while vstate["i"] <= item[0]:
                        vstep(4)
                    _av(item, p, po)
                # normalize. po bank0 rows = [s_h0 | o_h0], bank1 =
                # [s_h1 | o_h1]. recip must run at base partition 0 (custom
                # DVE ops misread at other bases); the muls mix partition
                # bases, allowed because one operand is PSUM. oT rows:
                # 0:64 = head 2p+1, 64:128 = head 2p (host swaps Wo rows).
                r_e = rpool.tile([64, 512], F32, name="r_e", tag="re")
                nc.vector.reciprocal_approx_fast(r_e[:], po[0:64, 0:512])
                nc.vector.tensor_mul(oT[64:128, isl], po[64:128, 0:512],
                                     r_e[:])
                r_o = rpool.tile([64, 512], F32, name="r_o", tag="ro")
                nc.vector.reciprocal_approx_fast(r_o[:], po[0:64, 512:1024])
                nc.vector.tensor_mul(oT[0:64, isl], po[64:128, 512:1024],
                                     r_o[:])
            # later pairs' q/k projections must be in place before their scores
            if p < PAIRS - 1:
                drain(qkgens[p + 1], 100)

        # ---------------- remaining out-projection blocks ----------------
        drain(late_filler, 100)
        for tb in range(4, TB):
            for _ in out_unit(tb):
                pass


def build_nc():
    nc = bacc.Bacc("TRN2", target_bir_lowering=False, debug=False,
                   num_devices=N_CORES)
    with tile.TileContext(nc) as tc:
        _emit(tc)
    nc.compile()
    return nc


def _get_compiled():
    if "nc" not in _CACHE:
        _CACHE["nc"] = build_nc()
    return _CACHE["nc"]


def make_in_maps(x_q, x_kv, pad_mask):
    ones = np.ones((128, PAIRS * CH), NPBF16)
    in_maps = []
    for c in range(N_CORES):
        b, g = divmod(c, 2)
        gs = slice(g * G, (g + 1) * G)
        mbias = np.where(pad_mask[b], np.float32(NEG), np.float32(0.0))
        in_maps.append({
            "xqT": np.ascontiguousarray(x_q[b].T).astype(NPBF16),
            "xkvT": np.ascontiguousarray(x_kv[b].T).astype(NPBF16),
            "wq": _W["q"][:, gs].astype(NPBF16),
            "wk": _W["k"][:, gs].astype(NPBF16),
            "wv": _W["v"][:, gs].astype(NPBF16),
            # oT pair blocks are [head 2p+1 | head 2p]; swap Wo rows
            "wo": _W["o"][gs, :].reshape(PAIRS, 2, CH, D)[:, ::-1]
                  .reshape(G, D).astype(NPBF16),
            "mb": np.ascontiguousarray(
                mbias.astype(np.float32).reshape(TB, 128).T),
            "vones": ones,
        })
    return in_maps


_W = {}


def kernel(x_q, x_kv, pad_mask, Wq, Wk, Wv, Wo, bo):
    global LAST_RESULTS
    x_q = np.asarray(x_q, dtype=np.float32)
    x_kv = np.asarray(x_kv, dtype=np.float32)
    pad_mask = np.asarray(pad_mask)
    _W["q"] = np.asarray(Wq, dtype=np.float32)
    _W["k"] = np.asarray(Wk, dtype=np.float32)
    _W["v"] = np.asarray(Wv, dtype=np.float32)
    _W["o"] = np.asarray(Wo, dtype=np.float32)
    bo = np.asarray(bo, dtype=np.float32)

    nc = _get_compiled()
    in_maps = make_in_maps(x_q, x_kv, pad_mask)
    res = bass_utils.run_bass_kernel_spmd(nc, in_maps, list(range(N_CORES)))
    LAST_RESULTS = res
    outp = np.zeros((B, T, D), np.float32)
    for b in range(B):
        outp[b] = (res.results[2 * b]["out"].astype(np.float32)
                   + res.results[2 * b + 1]["out"].astype(np.float32))
    outp += bo[None, None, :]
    return outp


# revision 21
# speedup vs baseline: 1.1629x; 1.0017x over previous
"""Multi-head attention, tensor-parallel over heads x data-parallel over batch.

8 NeuronCores: core c handles batch b=c//2, head-group g=c%2 (8 heads, 512 chans).
Each core computes its head-group's attention + partial output projection;
the two partials per batch are summed on the host (row-parallel Wo unshard).

v3: all matmul operands bf16 (fp32 PSUM accumulation), with the schedule built
around keeping TensorE dense while ScalarE streams the 64 exp ACTIVATEs:
  - input DMA split across three engine queues (sync / gpsimd / scalar)
  - pair-0 q/k projections k-streamed behind the DMA into two score-psum tiles
  - PE prewarmed with discarded matmuls so the HAM clock-gate opens early
  - v-projection + later pairs' q/k projections interleaved into attention
    as PE filler; out-projection token-blocks 0-3 fill pair-3's second half
  - AV stationaries are [ones | v] per head so the softmax denominators
    accumulate for free on partitions 0:64 of the same psum bank
  - normalization fully on DVE: reciprocal_approx_fast at base partition 0
    (required), cross-partition muls against the PSUM operand
"""

import numpy as np
import ml_dtypes

import concourse.bacc as bacc
import concourse.mybir as mybir
import concourse.tile as tile
from concourse import bass_utils

B = 4
T = 1024          # tokens (N = L)
D = 1024          # model dim
CH = 64           # channels per head
G = 512           # channels per head-group (8 heads)
SCALE = CH ** -0.5
NEG = -30000.0    # mask bias (exp(x + NEG) == 0)
F32 = mybir.dt.float32
BF16 = mybir.dt.bfloat16
NPBF16 = ml_dtypes.bfloat16

N_CORES = 8
KB = 8            # 128-row contraction blocks over D
TB = 8            # 128-token blocks
PAIRS = 4         # head pairs per core
VW = 256          # v-tile columns per head pair: [ones | v_even | ones | v_odd]
AV_LAG = 2        # software-pipeline depth: AV trails scores/exp by this many jb

LAST_RESULTS = None
_CACHE = {}


def _emit(tc):
    nc = tc.nc
    xqT = nc.dram_tensor("xqT", [D, T], BF16, kind="ExternalInput").ap()
    xkvT = nc.dram_tensor("xkvT", [D, T], BF16, kind="ExternalInput").ap()
    wq = nc.dram_tensor("wq", [D, G], BF16, kind="ExternalInput").ap()
    wk = nc.dram_tensor("wk", [D, G], BF16, kind="ExternalInput").ap()
    wv = nc.dram_tensor("wv", [D, G], BF16, kind="ExternalInput").ap()
    wo = nc.dram_tensor("wo", [G, D], BF16, kind="ExternalInput").ap()
    mb = nc.dram_tensor("mb", [128, TB], F32, kind="ExternalInput").ap()
    vones = nc.dram_tensor("vones", [128, PAIRS * CH], BF16,
                           kind="ExternalInput").ap()
    out = nc.dram_tensor("out", [T, D], BF16, kind="ExternalOutput").ap()

    Exp = mybir.ActivationFunctionType.Exp

    with (
        tc.tile_pool(name="wpool", bufs=1) as wpool,
        tc.tile_pool(name="xpool", bufs=1) as xpool,
        tc.tile_pool(name="actpool", bufs=1) as actpool,
        tc.tile_pool(name="etpool", bufs=16) as etpool,
        tc.tile_pool(name="rpool", bufs=2) as rpool,
        tc.tile_pool(name="opool", bufs=2) as opool,
        tc.tile_pool(name="psum_s", bufs=2, space="PSUM") as psum_s,
        tc.tile_pool(name="psum_o", bufs=1, space="PSUM") as psum_o,
        tc.tile_pool(name="psum_p", bufs=2, space="PSUM") as psum_p,
    ):
        # ------------- input DMA, split across engine queues -------------
        # critical path first on every queue (everything the pair-0 q/k
        # k-stream needs: xq, wq, wk, xkv); wv and wo trail afterwards.
        mask_t = wpool.tile([128, TB], F32, name="mask_t", tag="mask")
        nc.scalar.dma_start(mask_t[:], mb[:])
        ones_t = wpool.tile([128, PAIRS * CH], BF16, name="ones_t", tag="ones")
        nc.scalar.dma_start(ones_t[:], vones[:])

        xq_t = [xpool.tile([128, T], BF16, name=f"xq{k}", tag=f"xq{k}")
                for k in range(KB)]
        wq_t = [wpool.tile([128, G], BF16, name=f"wq{k}", tag=f"wq{k}")
                for k in range(KB)]
        # scalar queue: last two xq blocks, then wk, then (late) wo
        for k in (6, 7):
            nc.scalar.dma_start(xq_t[k][:], xqT[k * 128:(k + 1) * 128, :])
        wk_t = []
        for k in range(KB):
            t2 = wpool.tile([128, G], BF16, name=f"wk{k}", tag=f"wk{k}")
            nc.scalar.dma_start(t2[:], wk[k * 128:(k + 1) * 128, :])
            wk_t.append(t2)

        # sync queue: xkv, then (late) wv
        xkv_t = []
        for k in range(KB):
            t4 = xpool.tile([128, T], BF16, name=f"xkv{k}", tag=f"xkv{k}")
            nc.sync.dma_start(t4[:], xkvT[k * 128:(k + 1) * 128, :])
            xkv_t.append(t4)

        # gpsimd queue: xq blocks 0-5 interleaved with wq
        for k in range(6):
            nc.gpsimd.dma_start(xq_t[k][:], xqT[k * 128:(k + 1) * 128, :])
            nc.gpsimd.dma_start(wq_t[k][:], wq[k * 128:(k + 1) * 128, :])
        for k in (6, 7):
            nc.gpsimd.dma_start(wq_t[k][:], wq[k * 128:(k + 1) * 128, :])

        # trailing, non-critical: wv (gpsimd, which drains earliest), wo
        wv_t = []
        for k in range(KB):
            t5 = wpool.tile([128, G], BF16, name=f"wv{k}", tag=f"wv{k}")
            nc.gpsimd.dma_start(t5[:], wv[k * 128:(k + 1) * 128, :])
            wv_t.append(t5)
        wo_t = []
        for m in range(PAIRS):
            t6 = wpool.tile([128, D], BF16, name=f"wo{m}", tag=f"wo{m}")
            nc.scalar.dma_start(t6[:], wo[m * 128:(m + 1) * 128, :])
            wo_t.append(t6)

        qT_t = [actpool.tile([128, T], BF16, name=f"qT{m}", tag=f"qT{m}")
                for m in range(PAIRS)]
        kT_t = [actpool.tile([128, T], BF16, name=f"kT{m}", tag=f"kT{m}")
                for m in range(PAIRS)]
        v_t = [actpool.tile([128, PAIRS * VW], BF16, name=f"v{tb}",
                            tag=f"v{tb}") for tb in range(TB)]
        oT_t = [actpool.tile([128, T], BF16, name=f"oT{m}", tag=f"oT{m}")
                for m in range(PAIRS)]

        # dummy-matmul target: the po psum slot, overwritten (start=True)
        # by the first real AV matmul
        warm = psum_o.tile([128, 512], F32, name="warm", tag="po")
        # ones blocks of the v tiles, copied from SBUF (not strided DMA)
        for tb in range(TB):
            v4 = v_t[tb].rearrange("p (pb four c) -> p pb four c",
                                   four=4, c=CH)
            vo3 = ones_t.rearrange("p (pb c) -> p pb c", c=CH)
            nc.vector.tensor_copy(v4[:, :, 0, :], vo3[:])
            nc.vector.tensor_copy(v4[:, :, 2, :], vo3[:])

        # ------- pair-0 q/k projections, k-streamed behind the DMA -------
        psA = psum_s.tile([128, 1024], F32, name="psA", tag="sc")
        psB = psum_s.tile([128, 1024], F32, name="psB", tag="sc")
        for k in range(KB):
            for ic in range(2):
                csl = slice(ic * 512, (ic + 1) * 512)
                nc.tensor.matmul(psA[:, csl], wq_t[k][:, 0:128],
                                 xq_t[k][:, csl],
                                 start=(k == 0), stop=(k == KB - 1))
                nc.tensor.matmul(psB[:, csl], wk_t[k][:, 0:128],
                                 xkv_t[k][:, csl],
                                 start=(k == 0), stop=(k == KB - 1))
            # prewarm the PE while the prologue is DMA-bound: junk matmuls
            # into the warm slot open the HAM clock gate (~3.4us of activity)
            for _ in range(2):
                nc.tensor.matmul(warm[:], wq_t[k][:, 0:128],
                                 xq_t[k][:, 0:512], start=True, stop=True)
        nc.vector.tensor_copy(qT_t[0][:, 0:512], psA[:, 0:512])
        nc.vector.tensor_copy(qT_t[0][:, 512:1024], psA[:, 512:1024])
        nc.vector.tensor_copy(kT_t[0][:, 0:512], psB[:, 0:512])
        nc.vector.tensor_copy(kT_t[0][:, 512:1024], psB[:, 512:1024])

        # ---------------- filler units (run inside attention) ----------------
        def v_unit(tb):
            """v = xkv @ Wv for one token block: 8 MMs + 1 copy."""
            ps = psum_p.tile([128, 512], F32, name="ps_v", tag="pj")
            for k in range(KB):
                nc.tensor.matmul(
                    ps[:],
                    xkv_t[k][:, tb * 128:(tb + 1) * 128],
                    wv_t[k][:],
                    start=(k == 0),
                    stop=(k == KB - 1),
                )
                yield
            v4 = v_t[tb].rearrange("p (pb four c) -> p pb four c",
                                   four=4, c=CH)
            ps3 = ps.rearrange("p (pb two c) -> p pb two c", two=2, c=CH)
            nc.vector.tensor_copy(v4[:, :, 1:2, :], ps3[:, :, 0:1, :])
            nc.vector.tensor_copy(v4[:, :, 3:4, :], ps3[:, :, 1:2, :])
            yield

        def qk_unit(p, src, w_t, dst, ic):
            csl = slice(ic * 512, (ic + 1) * 512)
            ps = psum_p.tile([128, 512], F32, name="ps_qk", tag="pj")
            for k in range(KB):
                nc.tensor.matmul(
                    ps[:],
                    w_t[k][:, p * 128:(p + 1) * 128],
                    src[k][:, csl],
                    start=(k == 0),
                    stop=(k == KB - 1),
                )
                yield
            nc.vector.tensor_copy(dst[p][:, csl], ps[:])
            yield

        def out_unit(tb):
            """out[tb] = oT.T @ Wo: 8 MMs + copy + DMA."""
            tsl = slice(tb * 128, (tb + 1) * 128)
            ps = psum_s.tile([128, 1024], F32, name="ps_out", tag="sc")
            for ncx in range(2):
                nsl = slice(ncx * 512, (ncx + 1) * 512)
                for m in range(PAIRS):
                    nc.tensor.matmul(
                        ps[:, nsl],
                        oT_t[m][:, tsl],
                        wo_t[m][:, nsl],
                        start=(m == 0),
                        stop=(m == PAIRS - 1),
                    )
                    yield
            ot = opool.tile([128, 1024], BF16, name="ot", tag="ot")
            nc.vector.tensor_copy(ot[:], ps[:])
            # spread the tail output blocks over otherwise-idle queues
            eng = (nc.scalar, nc.scalar, nc.scalar, nc.scalar,
                   nc.sync, nc.gpsimd, nc.sync, nc.gpsimd)[tb]
            eng.dma_start(out[tsl, :], ot[:])
            yield

        def chain(gens):
            for g in gens:
                yield from g

        # v stream (AV deadlines) and one qk stream per later pair.
        # v units are tracked per-unit so an AV is never emitted before the
        # v tile it reads has been fully emitted (program order = dataflow).
        vunits = [v_unit(tb) for tb in range(TB)]
        vstate = {"i": 0}

        def vstep(n):
            k = 0
            while k < n and vstate["i"] < TB:
                if next(vunits[vstate["i"]], "done") == "done":
                    vstate["i"] += 1
                else:
                    k += 1

        qkgens = {
            p: chain([qk_unit(p, src, w_t, dst, ic)
                      for src, w_t, dst in ((xkv_t, wk_t, kT_t),
                                            (xq_t, wq_t, qT_t))
                      for ic in range(2)])
            for p in range(1, PAIRS)
        }
        # out-projection blocks 0-3 only need ih=0 columns of oT: they fill
        # pair 3's second-half attention
        late_filler = chain([out_unit(tb) for tb in range(4)])

        def drain(gen, n):
            for _ in range(n):
                if next(gen, "done") == "done":
                    return

        # ---------------- attention ----------------
        def _av(item, p, po):
            jb, et = item
            for h in (0, 1):
                # head 2p+h stationary [ones | v]: sums land on partitions
                # 0:64, o on 64:128; one accumulation group per psum bank
                csl = slice(p * VW + h * 128, p * VW + h * 128 + 128)
                nc.tensor.matmul(
                    po[:, h * 512:(h + 1) * 512],
                    v_t[jb][:, csl],
                    et[:, h * 512:(h + 1) * 512],
                    start=(jb == 0),
                    stop=(jb == TB - 1),
                )

        for p in range(PAIRS):
            oT = oT_t[p]
            for ih in range(2):
                isl = slice(ih * 512, (ih + 1) * 512)
                po = psum_o.tile([128, 1024], F32, name="po", tag="po")
                pend = []
                for jb in range(TB):
                    jsl = slice(jb * 128, (jb + 1) * 128)
                    pss = psum_s.tile([128, 1024], F32, name="ps_s", tag="sc")
                    for h in (0, 1):
                        hsl = slice(h * 64, (h + 1) * 64)
                        nc.tensor.matmul(
                            pss[:, h * 512:(h + 1) * 512],
                            kT_t[p][hsl, jsl],
                            qT_t[p][hsl, isl],
                        )
                    # one exp for both heads; mask bias is per-partition (= j)
                    et = etpool.tile([128, 1024], BF16, name="et", tag="et")
                    nc.scalar.activation(et[:], pss[:], Exp,
                                         bias=mask_t[:, jb:jb + 1],
                                         scale=SCALE)
                    pend.append((jb, et))
                    if len(pend) > AV_LAG:
                        item = pend.pop(0)
                        while vstate["i"] <= item[0]:
                            vstep(4)
                        _av(item, p, po)
                    vstep(3)
                    if p == PAIRS - 1 and ih == 1:
                        drain(late_filler, 2)
                    elif p < PAIRS - 1:
                        drain(qkgens[p + 1], 2)
                while pend:
                    item = pend.pop(0)
                    while vstate["i"] <= item[0]:
                        vstep(4)
                    _av(item, p, po)
                # normalize. po bank0 rows = [s_h0 | o_h0], bank1 =
                # [s_h1 | o_h1]. recip must run at base partition 0 (custom
                # DVE ops misread at other bases); the muls mix partition
                # bases, allowed because one operand is PSUM. oT rows:
                # 0:64 = head 2p+1, 64:128 = head 2p (host swaps Wo rows).
                r_e = rpool.tile([64, 512], F32, name="r_e", tag="re")
                nc.vector.reciprocal_approx_fast(r_e[:], po[0:64, 0:512])
                nc.vector.tensor_mul(oT[64:128, isl], po[64:128, 0:512],
                                     r_e[:])
                r_o = rpool.tile([64, 512], F32, name="r_o", tag="ro")
                nc.vector.reciprocal_approx_fast(r_o[:], po[0:64, 512:1024])
                nc.vector.tensor_mul(oT[0:64, isl], po[64:128, 512:1024],
                                     r_o[:])
            # later pairs' q/k projections must be in place before their scores
            if p < PAIRS - 1:
                drain(qkgens[p + 1], 100)

        # ---------------- remaining out-projection blocks ----------------
        drain(late_filler, 100)
        for tb in range(4, TB):
            for _ in out_unit(tb):
                pass


def build_nc():
    nc = bacc.Bacc("TRN2", target_bir_lowering=False, debug=False,
                   num_devices=N_CORES)
    with tile.TileContext(nc) as tc:
        _emit(tc)
    nc.compile()
    return nc


def _get_compiled():
    if "nc" not in _CACHE:
        _CACHE["nc"] = build_nc()
    return _CACHE["nc"]


def make_in_maps(x_q, x_kv, pad_mask):
    ones = np.ones((128, PAIRS * CH), NPBF16)
    in_maps = []
    for c in range(N_CORES):
        b, g = divmod(c, 2)
        gs = slice(g * G, (g + 1) * G)
        mbias = np.where(pad_mask[b], np.float32(NEG), np.float32(0.0))
        in_maps.append({
            "xqT": np.ascontiguousarray(x_q[b].T).astype(NPBF16),
            "xkvT": np.ascontiguousarray(x_kv[b].T).astype(NPBF16),
            "wq": _W["q"][:, gs].astype(NPBF16),
            "wk": _W["k"][:, gs].astype(NPBF16),
            "wv": _W["v"][:, gs].astype(NPBF16),
            # oT pair blocks are [head 2p+1 | head 2p]; swap Wo rows
            "wo": _W["o"][gs, :].reshape(PAIRS, 2, CH, D)[:, ::-1]
                  .reshape(G, D).astype(NPBF16),
            "mb": np.ascontiguousarray(
                mbias.astype(np.float32).reshape(TB, 128).T),
            "vones": ones,
        })
    return in_maps


_W = {}


def kernel(x_q, x_kv, pad_mask, Wq, Wk, Wv, Wo, bo):
    global LAST_RESULTS
    x_q = np.asarray(x_q, dtype=np.float32)
    x_kv = np.asarray(x_kv, dtype=np.float32)
    pad_mask = np.asarray(pad_mask)
    _W["q"] = np.asarray(Wq, dtype=np.float32)
    _W["k"] = np.asarray(Wk, dtype=np.float32)
    _W["v"] = np.asarray(Wv, dtype=np.float32)
    _W["o"] = np.asarray(Wo, dtype=np.float32)
    bo = np.asarray(bo, dtype=np.float32)

    nc = _get_compiled()
    in_maps = make_in_maps(x_q, x_kv, pad_mask)
    res = bass_utils.run_bass_kernel_spmd(nc, in_maps, list(range(N_CORES)))
    LAST_RESULTS = res
    outp = np.zeros((B, T, D), np.float32)
    for b in range(B):
        outp[b] = (res.results[2 * b]["out"].astype(np.float32)
                   + res.results[2 * b + 1]["out"].astype(np.float32))
    outp += bo[None, None, :]
    return outp
